# revision 8
# baseline (speedup 1.0000x reference)
"""BitLinear (2-bit packed ternary-ish weights) on 8 Trainium2 NeuronCores.

y = round_int8(x * 127/amax_row) @ unpack(weight_packed).T / (ws * 127/amax_row) + bias

Sharding: data-parallel over the flattened token dim (16384 rows -> 2048
rows/core). The packed weight is tiny; it is unpacked host-side to bf16
(values {-1,0,1,2}, exact in bf16) and replicated to all cores.

On-device math is bit-faithful to the jax reference:
  - absmax reduce + clip:   exact f32 ops
  - scale = 127/amax:       DVE IEEE f32 divide
  - xq = round(x*scale):    DVE two-stage (mult, +1.5*2^23) == f32 mult-round
                            then round-half-to-even; ints <=127 exact in bf16
  - matmul:                 bf16 PE with f32 PSUM accumulation — products and
                            partial sums are integers < 2^24 => exact
  - dequant:                psum / (ws*scale) per-row DVE divide, + bias
"""

from contextlib import ExitStack

import numpy as np
import ml_dtypes

import concourse.bass as bass
import concourse.mybir as mybir
import concourse.tile as tile
from concourse import bacc
from concourse.bass_utils import run_bass_kernel_spmd
from concourse.masks import make_identity

P = 128
D = 2048               # in_features
O = 2048               # out_features (4 * 512 packed rows)
N_CORES = 8
B, S = 4, 4096
M_TOTAL = B * S        # 16384
M_CORE = M_TOTAL // N_CORES   # 2048
NK = D // P            # 16 contraction blocks
O_CHUNK = 512          # one PSUM bank of f32
N_OCH = O // O_CHUNK   # 4
MAGIC = 12582912.0     # 1.5 * 2^23 — f32 add forces round-half-to-even to int
QP = 127.0


def build_nc(m_core=M_CORE, repeats=1):
    m_tiles = m_core // P
    nc = bacc.Bacc(None)
    x = nc.declare_dram_parameter("x", [m_core, D], mybir.dt.float32, isOutput=False)
    wT = nc.declare_dram_parameter("wT", [D, O], mybir.dt.bfloat16, isOutput=False)
    bias = nc.declare_dram_parameter("bias", [O], mybir.dt.float32, isOutput=False)
    ws = nc.declare_dram_parameter("ws", [1], mybir.dt.float32, isOutput=False)
    y = nc.declare_dram_parameter("y", [m_core, O], mybir.dt.float32, isOutput=True)

    with ExitStack() as ctx:
        tc = ctx.enter_context(tile.TileContext(nc))
        consts = ctx.enter_context(tc.tile_pool(name="consts", bufs=1))
        xpool = ctx.enter_context(tc.tile_pool(name="xin", bufs=3))
        qpool = ctx.enter_context(tc.tile_pool(name="quant", bufs=2))
        tppool = ctx.enter_context(tc.tile_pool(name="xqt", bufs=32))
        spool = ctx.enter_context(tc.tile_pool(name="stats", bufs=4))
        opool = ctx.enter_context(tc.tile_pool(name="yout", bufs=3))
        pst = ctx.enter_context(tc.tile_pool(name="pst", bufs=4, space="PSUM"))
        psy = ctx.enter_context(tc.tile_pool(name="psy", bufs=1, space="PSUM"))

        ident = consts.tile([P, P], mybir.dt.bfloat16)
        make_identity(nc, ident[:])
        bias_sb = consts.tile([P, O], mybir.dt.float32)
        nc.sync.dma_start(bias_sb[:], bias[None, :].to_broadcast((P, O)))
        ws_sb = consts.tile([P, 1], mybir.dt.float32)
        nc.sync.dma_start(ws_sb[:], ws[None, :].to_broadcast((P, 1)))
        w_sb = consts.tile([P, NK, O], mybir.dt.bfloat16)
        nc.sync.dma_start(w_sb[:], wT.rearrange("(k p) o -> p k o", p=P))

        x3 = x.rearrange("(t p) d -> t p d", p=P)
        y3 = y.rearrange("(t p) o -> t p o", p=P)

        for _ in range(repeats):
            for t in range(m_tiles):
                xt = xpool.tile([P, D], mybir.dt.float32, tag="xin")
                nc.sync.dma_start(xt[:], x3[t])

                amax = spool.tile([P, 1], mybir.dt.float32, tag="amax")
                nc.vector.reduce_max(
                    amax[:], xt[:], axis=mybir.AxisListType.X,
                    apply_absolute_value=True,
                )
                nc.vector.tensor_scalar_max(amax[:], amax[:], 1e-5)
                # scl = 127 * (1/amax); HW reciprocal is IEEE 1/x, so scl is
                # within 1 ulp of the reference's fl(127/amax)
                ramax = spool.tile([P, 1], mybir.dt.float32, tag="ramax")
                nc.vector.reciprocal(ramax[:], amax[:])
                scl = spool.tile([P, 1], mybir.dt.float32, tag="scl")
                nc.vector.tensor_scalar_mul(scl[:], ramax[:], QP)
                den = spool.tile([P, 1], mybir.dt.float32, tag="den")
                nc.vector.tensor_tensor(
                    den[:], ws_sb[:], scl[:], mybir.AluOpType.mult
                )
                rden = spool.tile([P, 1], mybir.dt.float32, tag="rden")
                nc.vector.reciprocal(rden[:], den[:])

                # xq = round_half_even(x * scale), exact ints in bf16
                t1 = qpool.tile([P, D], mybir.dt.float32, tag="t1")
                nc.vector.tensor_scalar(
                    t1[:], xt[:], scl[:], MAGIC,
                    op0=mybir.AluOpType.mult, op1=mybir.AluOpType.add,
                )
                xq = qpool.tile([P, D], mybir.dt.bfloat16, tag="xq")
                nc.scalar.activation(
                    xq[:], t1[:], mybir.ActivationFunctionType.Copy,
                    bias=-MAGIC, scale=1.0,
                )

                # transpose xq -> 16 tiles of [128 d, 128 m]
                xqT = []
                for k in range(NK):
                    ptile = pst.tile([P, P], mybir.dt.bfloat16, tag="pst")
                    nc.tensor.transpose(ptile[:], xq[:, bass.ts(k, P)], ident[:])
                    st = tppool.tile([P, P], mybir.dt.bfloat16, tag="xqT")
                    nc.scalar.copy(st[:], ptile[:])
                    xqT.append(st)

                # y[m, o] += xqT.T @ wT, accumulated over 16 d-blocks
                ys = [
                    psy.tile([P, O_CHUNK], mybir.dt.float32,
                             tag=f"psy{j}", name=f"psy{j}")
                    for j in range(N_OCH)
                ]
                for k in range(NK):
                    for j in range(N_OCH):
                        nc.tensor.matmul(
                            ys[j][:], xqT[k][:],
                            w_sb[:, k, bass.ts(j, O_CHUNK)],
                            start=(k == 0), stop=(k == NK - 1),
                        )

                yt = opool.tile([P, O], mybir.dt.float32, tag="yt")
                for j in range(N_OCH):
                    nc.vector.tensor_scalar(
                        yt[:, bass.ts(j, O_CHUNK)], ys[j][:], rden[:], None,
                        op0=mybir.AluOpType.mult,
                    )
                nc.vector.tensor_tensor(
                    yt[:], yt[:], bias_sb[:], mybir.AluOpType.add
                )
                nc.sync.dma_start(y3[t], yt[:])
    nc.finalize()
    return nc


def unpack_weights_host(weight_packed):
    """[512, 2048] int32 packed -> [2048 in, 2048 out] bf16 transposed weight."""
    wp = np.asarray(weight_packed)
    parts = [((wp >> (2 * i)) & 3) for i in range(4)]
    w = np.concatenate(parts, axis=0).astype(np.float32) - 1.0   # [out, in]
    return np.ascontiguousarray(w.T).astype(ml_dtypes.bfloat16)  # [in, out]


_NC_CACHE = {}


def _get_nc():
    if "nc" not in _NC_CACHE:
        _NC_CACHE["nc"] = build_nc()
    return _NC_CACHE["nc"]


def kernel(x, weight_packed, weight_scale, bias):
    xf = np.ascontiguousarray(np.asarray(x, dtype=np.float32).reshape(M_TOTAL, D))
    wT = unpack_weights_host(weight_packed)
    bias_np = np.ascontiguousarray(np.asarray(bias, dtype=np.float32))
    ws_np = np.ascontiguousarray(np.asarray(weight_scale, dtype=np.float32))

    in_maps = [
        {
            "x": xf[i * M_CORE:(i + 1) * M_CORE],
            "wT": wT,
            "bias": bias_np,
            "ws": ws_np,
        }
        for i in range(N_CORES)
    ]
    res = run_bass_kernel_spmd(_get_nc(), in_maps, list(range(N_CORES))).results
    y = np.concatenate([res[i]["y"] for i in range(N_CORES)], axis=0)
    return np.ascontiguousarray(y.reshape(B, S, O))
